# revision 1
# baseline (speedup 1.0000x reference)
"""MoE (7 routed top-2 + 1 shared expert) Trainium2 kernel, 8-core data-parallel.

Strategy: data-parallel over tokens (1024 tokens/core), all weights replicated.
Per core: exact fp32 gate + routing (top-2 mask * softmax), then per expert:
fc matmul (float32r), exact-erf GELU on ScalarE, proj matmul (float32r) with
output in token-partition layout, combine scaled by routing weights into an
SBUF accumulator, single DMA store. Gate runs in fp32 so top-2 selection
matches the reference bit-for-bit; the big MLP matmuls use float32r
(TF32-like, 4x the fp32 PE rate, ~2e-4 rel err).
"""

import sys

for _p in ("/opt/trn_rl_repo", "/root/.axon_site/_ro/trn_rl_repo"):
    if _p not in sys.path:
        sys.path.append(_p)

import numpy as np

import concourse.bass as bass
import concourse.mybir as mybir
from concourse import bacc
from concourse.masks import make_identity
from concourse.tile import TileContext

F32 = mybir.dt.float32
F32R = mybir.dt.float32r

N_CORES = 8
B, T, C = 4, 2048, 1024
H = 4 * C
NE = 8          # 7 routed + 1 shared
NR = 7          # routed experts
NT = B * T // N_CORES   # tokens per core = 1024
NTP = NT // 128         # token tiles per core = 8
NKC = C // 128          # contraction tiles over C = 8
NHM = H // 128          # H tiles = 32
BLK = 512               # token block
NBLK = NT // BLK        # 2 blocks per core
NEG_INF = -1.0e30


def build_moe_nc(repeat: int = 1):
    nc = bacc.Bacc("TRN2", target_bir_lowering=False, debug=False, num_devices=N_CORES)

    x_d = nc.declare_dram_parameter("x", [NT, C], F32, isOutput=False)
    gw_d = nc.declare_dram_parameter("gate_w", [NR, C], F32, isOutput=False)
    lb_d = nc.declare_dram_parameter("lb_bias", [NR], F32, isOutput=False)
    swfc_d = nc.declare_dram_parameter("shared_wfc", [C, H], F32, isOutput=False)
    swpj_d = nc.declare_dram_parameter("shared_wproj", [H, C], F32, isOutput=False)
    rwfc_d = nc.declare_dram_parameter("routed_wfc", [NR, C, H], F32, isOutput=False)
    rwpj_d = nc.declare_dram_parameter("routed_wproj", [NR, H, C], F32, isOutput=False)
    y_d = nc.declare_dram_parameter("y", [NT, C], F32, isOutput=True)

    def emit(tc):
        _emit_body(nc, tc, x_d, gw_d, lb_d, swfc_d, swpj_d, rwfc_d, rwpj_d, y_d)

    with TileContext(nc) as tc:
        if repeat == 1:
            emit(tc)
        else:
            with tc.For_i(0, repeat, 1):
                emit(tc)
    nc.compile()
    return nc


def _emit_body(nc, tc, x_d, gw_d, lb_d, swfc_d, swpj_d, rwfc_d, rwpj_d, y_d):
    if True:
        with (
            tc.tile_pool(name="const", bufs=1) as cpool,
            tc.tile_pool(name="xtr", bufs=1) as xtrpool,
            tc.tile_pool(name="yacc", bufs=1) as ypool,
        ):
            ident = cpool.tile([128, 128], F32)
            make_identity(nc, ident[:])

            xTr = xtrpool.tile([128, NKC, NT], F32R)      # x^T, fp32r, all tokens
            y_acc = ypool.tile([128, NTP, C], F32)        # output accumulator [tok-p, tp, C]
            cw = cpool.tile([128, NTP, NR], F32)          # combine weights per token

            # ---------------- stage 1: transpose x, gate, routing ----------------
            with (
                tc.tile_pool(name="xt", bufs=1) as xtpool,
                tc.tile_pool(name="stage1", bufs=2) as s1pool,
                tc.tile_pool(name="psum_t", bufs=4, space="PSUM") as tpsum,
                tc.tile_pool(name="psum_g", bufs=2, space="PSUM") as gpsum,
            ):
                xT = xtpool.tile([128, NKC, NT], F32)

                # transpose x into xT (and round into xTr)
                for tp in range(NTP):
                    x_sb = s1pool.tile([128, C], F32, tag="x_stage")
                    nc.sync.dma_start(out=x_sb[:], in_=x_d[tp * 128:(tp + 1) * 128, :])
                    for kc in range(NKC):
                        pt = tpsum.tile([128, 128], F32, tag="tps")
                        nc.tensor.transpose(pt[:], x_sb[:, kc * 128:(kc + 1) * 128], ident[:])
                        nc.vector.tensor_copy(xT[:, kc, tp * 128:(tp + 1) * 128], pt[:])
                        nc.scalar.copy(xTr[:, kc, tp * 128:(tp + 1) * 128], pt[:])

                # gate weights transposed: gwT[128, kc, NR]
                gw_sb = cpool.tile([NR, C], F32)
                nc.sync.dma_start(out=gw_sb[:], in_=gw_d[:, :])
                ident7 = cpool.tile([NR, NR], F32)
                make_identity(nc, ident7[:])
                gwT = cpool.tile([128, NKC, NR], F32)
                for kc in range(NKC):
                    pt = tpsum.tile([128, NR], F32, tag="tps")
                    nc.tensor.transpose(pt[:], gw_sb[:, kc * 128:(kc + 1) * 128], ident7[:])
                    nc.vector.tensor_copy(gwT[:, kc, :], pt[:])

                # lb_bias broadcast to all partitions
                lbb = cpool.tile([128, NR], F32)
                nc.sync.dma_start(out=lbb[:], in_=lb_d[:].partition_broadcast(128))

                # gate logits + routing per token tile
                for tp in range(NTP):
                    pl = gpsum.tile([128, NR], F32, tag="plog")
                    for kc in range(NKC):
                        nc.tensor.matmul(
                            pl[:],
                            xT[:, kc, tp * 128:(tp + 1) * 128],
                            gwT[:, kc, :],
                            start=(kc == 0),
                            stop=(kc == NKC - 1),
                        )
                    logit = s1pool.tile([128, NR], F32, tag="logit")
                    nc.vector.tensor_copy(logit[:], pl[:])

                    sel = s1pool.tile([128, NR], F32, tag="sel")
                    nc.vector.tensor_add(sel[:], logit[:], lbb[:])

                    top8 = s1pool.tile([128, 8], F32, tag="top8")
                    nc.vector.memset(top8[:], NEG_INF)
                    nc.vector.tensor_copy(top8[:, 0:NR], sel[:])
                    mx8 = s1pool.tile([128, 8], F32, tag="mx8")
                    nc.vector.max(mx8[:], top8[:])

                    mask = s1pool.tile([128, NR], F32, tag="mask")
                    nc.vector.tensor_scalar(
                        mask[:], sel[:], mx8[:, 1:2], None, op0=mybir.AluOpType.is_ge
                    )

                    nmax = s1pool.tile([128, 1], F32, tag="nmax")
                    nc.vector.reduce_max(nmax[:], logit[:], axis=mybir.AxisListType.X, negate=True)
                    expo = s1pool.tile([128, NR], F32, tag="expo")
                    ssum = s1pool.tile([128, 1], F32, tag="ssum")
                    nc.scalar.activation(
                        expo[:], logit[:], mybir.ActivationFunctionType.Exp,
                        bias=nmax[:], scale=1.0, accum_out=ssum[:],
                    )
                    rs = s1pool.tile([128, 1], F32, tag="rs")
                    nc.vector.reciprocal(rs[:], ssum[:])
                    nc.vector.tensor_mul(expo[:], expo[:], mask[:])
                    nc.vector.tensor_scalar_mul(cw[:, tp, :], expo[:], rs[:])

            # ---------------- stage 2: experts ----------------
            with (
                tc.tile_pool(name="ht", bufs=1) as htpool,
                tc.tile_pool(name="wfc", bufs=2) as wfcpool,
                tc.tile_pool(name="wpj", bufs=10) as wpjpool,
                tc.tile_pool(name="drain", bufs=4) as drpool,
                tc.tile_pool(name="psum_fc", bufs=4, space="PSUM") as fcpsum,
                tc.tile_pool(name="psum_pj", bufs=4, space="PSUM") as pjpsum,
            ):
                hT = htpool.tile([128, NHM, BLK], F32R)

                # shared expert first (e == NE-1): plain copy into y_acc.
                for e in [NE - 1] + list(range(NR)):
                    shared = e == NE - 1
                    for blk in range(NBLK):
                        # ---- fc: hT[h, tok_blk] = gelu(wfc^T x^T) ----
                        for ch in range(NHM // 4):   # H chunks of 512 cols
                            wfc_sb = wfcpool.tile([128, NKC, 512], F32R, tag="wfc")
                            if shared:
                                src = swfc_d[:, ch * 512:(ch + 1) * 512]
                            else:
                                src = rwfc_d[e, :, ch * 512:(ch + 1) * 512]
                            nc.sync.dma_start(
                                out=wfc_sb[:],
                                in_=src.rearrange("(kc p) m -> p kc m", p=128).bitcast(F32R),
                            )
                            for h4 in range(4):
                                hm = ch * 4 + h4
                                ph = fcpsum.tile([128, BLK], F32, tag="fc")
                                for kc in range(NKC):
                                    nc.tensor.matmul(
                                        ph[:],
                                        wfc_sb[:, kc, h4 * 128:(h4 + 1) * 128],
                                        xTr[:, kc, blk * BLK:(blk + 1) * BLK],
                                        start=(kc == 0),
                                        stop=(kc == NKC - 1),
                                    )
                                nc.scalar.activation(
                                    hT[:, hm, :], ph[:], mybir.ActivationFunctionType.Gelu
                                )

                        # ---- proj: y[tok_blk, C] += cw_e * (hT^T wproj) ----
                        for nh in range(2):          # C halves of 512
                            pys = [
                                pjpsum.tile([128, 512], F32, tag="pj", name=f"py{i}")
                                for i in range(4)
                            ]
                            for kh in range(NHM):
                                wpj_sb = wpjpool.tile([128, 512], F32R, tag="wpj")
                                if shared:
                                    srcp = swpj_d[kh * 128:(kh + 1) * 128,
                                                  nh * 512:(nh + 1) * 512]
                                else:
                                    srcp = rwpj_d[e, kh * 128:(kh + 1) * 128,
                                                  nh * 512:(nh + 1) * 512]
                                nc.sync.dma_start(out=wpj_sb[:], in_=srcp.bitcast(F32R))
                                for tm in range(4):  # token sub-tiles in block
                                    nc.tensor.matmul(
                                        pys[tm][:],
                                        hT[:, kh, tm * 128:(tm + 1) * 128],
                                        wpj_sb[:],
                                        start=(kh == 0),
                                        stop=(kh == NHM - 1),
                                    )
                            for tm in range(4):
                                tp = blk * 4 + tm
                                ys = y_acc[:, tp, nh * 512:(nh + 1) * 512]
                                if shared:
                                    nc.vector.tensor_copy(ys, pys[tm][:])
                                else:
                                    tmp = drpool.tile([128, 512], F32, tag="dr")
                                    nc.vector.tensor_scalar(
                                        tmp[:], pys[tm][:], cw[:, tp, e:e + 1], None,
                                        op0=mybir.AluOpType.mult,
                                    )
                                    nc.vector.tensor_add(ys, ys, tmp[:])

            # ---------------- stage 3: store ----------------
            for tp in range(NTP):
                nc.sync.dma_start(
                    out=y_d[tp * 128:(tp + 1) * 128, :], in_=y_acc[:, tp, :]
                )


_NC_CACHE = None


def _get_nc():
    global _NC_CACHE
    if _NC_CACHE is None:
        _NC_CACHE = build_moe_nc()
    return _NC_CACHE


def kernel(**inputs) -> np.ndarray:
    from concourse.bass_utils import run_bass_kernel_spmd

    x = np.ascontiguousarray(np.asarray(inputs["x"], dtype=np.float32))
    shared = {
        "gate_w": np.ascontiguousarray(np.asarray(inputs["gate_w"], dtype=np.float32)),
        "lb_bias": np.ascontiguousarray(np.asarray(inputs["lb_bias"], dtype=np.float32)),
        "shared_wfc": np.ascontiguousarray(np.asarray(inputs["shared_wfc"], dtype=np.float32)),
        "shared_wproj": np.ascontiguousarray(np.asarray(inputs["shared_wproj"], dtype=np.float32)),
        "routed_wfc": np.ascontiguousarray(np.asarray(inputs["routed_wfc"], dtype=np.float32)),
        "routed_wproj": np.ascontiguousarray(np.asarray(inputs["routed_wproj"], dtype=np.float32)),
    }
    xt = x.reshape(-1, C)
    in_maps = [
        {"x": np.ascontiguousarray(xt[c * NT:(c + 1) * NT]), **shared}
        for c in range(N_CORES)
    ]
    nc = _get_nc()
    res = run_bass_kernel_spmd(nc, in_maps, list(range(N_CORES)))
    out = np.concatenate([res.results[c]["y"] for c in range(N_CORES)], axis=0)
    return out.reshape(B, T, C).astype(np.float32)

